# revision 5
# baseline (speedup 1.0000x reference)
"""Trainium2 Bass kernel for nn_MoELayer_1073741824588.

Strategy (self-contained; shapes hardcoded for N=8192, D=1024, E=8 experts,
top-2 routing, 4 "fractal" experts with hidden 2048 + 4 plain SwiGLU experts
with hidden 4096):

  * Host (numpy): gate (softmax + top-2 + renorm), RMS norm, token routing.
  * The expert FLOPs are decomposed into 24 uniform jobs: each expert's
    hidden dim is split into 1024-wide chunks (fractal: 2 chunks, plain: 4),
    and each job processes all tokens routed to that expert. Jobs are
    greedily balanced 3-per-core across the 8 NeuronCores.
  * Device (Bass/Tile, SPMD on 8 cores): each core runs 3 generic SwiGLU
    chunk units:  out = W2c @ (silu(W1c @ X) * (W3c @ X))  with
    W1c/W3c: [1024h, 1024d], W2c: [1024d, 1024h], X: [1024d, T_PAD tokens].
    Matmuls run in float32r (single-pass, fp32 accumulate). gamma (fractal
    residual scale) is folded into W2c on the host.
  * Host: combine — scatter-add cw-weighted unit outputs plus the fractal
    residual terms cw*(gamma*yn + x).
"""

import numpy as np
import os
import sys

for _p in ("/opt/trn_rl_repo",):
    if _p not in sys.path:
        sys.path.insert(0, _p)

import concourse.bacc as bacc
import concourse.mybir as mybir
import concourse.tile as tile
from concourse import bass_utils

D = 1024
N_TOK = 8192
E = 8
F = 4          # fractal experts (hidden 2*D)
P = 4          # plain experts (hidden 4*D)
TOPK = 2
EPS = 1e-6
HC = 1024      # hidden chunk per job
T_PAD = 2304   # padded token capacity per job (max observed count ~2175)
N_CORES = 8
UPC = 3        # units per core
TT = 512       # token tile (matmul moving free dim)
F32 = mybir.dt.float32
F32R = mybir.dt.float32r

_COMPILED = None
_LAST_RESULTS = None


def _build_program():
    """One SPMD program: 3 generic SwiGLU-chunk units of fixed shape."""
    nc = bacc.Bacc("TRN2", target_bir_lowering=False, debug=False)

    w1t = nc.dram_tensor("w1t", [UPC, D, HC], F32, kind="ExternalInput")
    w3t = nc.dram_tensor("w3t", [UPC, D, HC], F32, kind="ExternalInput")
    w2t = nc.dram_tensor("w2t", [UPC, HC, D], F32, kind="ExternalInput")
    xt = nc.dram_tensor("xt", [UPC, D, T_PAD], F32, kind="ExternalInput")
    out = nc.dram_tensor("out", [UPC, D, T_PAD], F32, kind="ExternalOutput")

    KD = D // 128    # 8 k-chunks over model dim
    MH = HC // 128   # 8 h-subchunks per unit
    n_tiles = (T_PAD + TT - 1) // TT

    with tile.TileContext(nc) as tc:
        with (
            tc.tile_pool(name="wpool", bufs=1) as wpool,
            tc.tile_pool(name="xpool", bufs=2) as xpool,
            tc.tile_pool(name="hpool", bufs=2) as hpool,
            tc.tile_pool(name="spool", bufs=3) as spool,
            tc.tile_pool(name="opool", bufs=3) as opool,
            tc.tile_pool(name="ps1", bufs=2, space="PSUM") as pp1,
            tc.tile_pool(name="ps3", bufs=2, space="PSUM") as pp3,
            tc.tile_pool(name="pso", bufs=2, space="PSUM") as ppo,
        ):
            for u in range(UPC):
                w1sb = []
                w3sb = []
                w2sb = []
                for k in range(KD):
                    t = wpool.tile([128, HC], F32R, tag=f"w1_{k}")
                    nc.sync.dma_start(t[:], w1t[u, k * 128:(k + 1) * 128, :].bitcast(F32R))
                    w1sb.append(t)
                for k in range(KD):
                    t = wpool.tile([128, HC], F32R, tag=f"w3_{k}")
                    nc.sync.dma_start(t[:], w3t[u, k * 128:(k + 1) * 128, :].bitcast(F32R))
                    w3sb.append(t)
                for m in range(MH):
                    t = wpool.tile([128, D], F32R, tag=f"w2_{m}")
                    nc.sync.dma_start(t[:], w2t[u, m * 128:(m + 1) * 128, :].bitcast(F32R))
                    w2sb.append(t)

                for ti in range(n_tiles):
                    t0 = ti * TT
                    tt = min(TT, T_PAD - t0)

                    xsb = []
                    for k in range(KD):
                        t = xpool.tile([128, TT], F32R, tag=f"x_{k}")
                        nc.sync.dma_start(
                            t[:, :tt],
                            xt[u, k * 128:(k + 1) * 128, t0:t0 + tt].bitcast(F32R),
                        )
                        xsb.append(t)

                    hf = []
                    for m in range(MH):
                        ps1 = pp1.tile([128, TT], F32, tag="ps1")
                        ps3 = pp3.tile([128, TT], F32, tag="ps3")
                        for k in range(KD):
                            nc.tensor.matmul(
                                ps1[:, :tt],
                                w1sb[k][:, m * 128:(m + 1) * 128],
                                xsb[k][:, :tt],
                                start=(k == 0),
                                stop=(k == KD - 1),
                            )
                        for k in range(KD):
                            nc.tensor.matmul(
                                ps3[:, :tt],
                                w3sb[k][:, m * 128:(m + 1) * 128],
                                xsb[k][:, :tt],
                                start=(k == 0),
                                stop=(k == KD - 1),
                            )
                        sl = spool.tile([128, TT], F32, tag="silu")
                        nc.scalar.activation(
                            sl[:, :tt], ps1[:, :tt],
                            mybir.ActivationFunctionType.Silu,
                        )
                        h = hpool.tile([128, TT], F32R, tag=f"hf_{m}")
                        nc.vector.tensor_mul(h[:, :tt], sl[:, :tt], ps3[:, :tt])
                        hf.append(h)

                    for d in range(KD):
                        pso = ppo.tile([128, TT], F32, tag="pso")
                        for m in range(MH):
                            nc.tensor.matmul(
                                pso[:, :tt],
                                w2sb[m][:, d * 128:(d + 1) * 128],
                                hf[m][:, :tt],
                                start=(m == 0),
                                stop=(m == MH - 1),
                            )
                        ob = opool.tile([128, TT], F32, tag="ob")
                        nc.vector.tensor_copy(ob[:, :tt], pso[:, :tt])
                        nc.sync.dma_start(
                            out[u, d * 128:(d + 1) * 128, t0:t0 + tt], ob[:, :tt]
                        )

    nc.compile()
    return nc


def _get_compiled():
    global _COMPILED
    if _COMPILED is None:
        _COMPILED = _build_program()
    return _COMPILED


def _np_silu(v):
    return v / (1.0 + np.exp(-v))


def kernel(x, Wg, rms_w, gamma, w1f, w3f, w2f, w1p, w3p, w2p):
    x = np.ascontiguousarray(np.asarray(x, np.float32))
    Wg = np.asarray(Wg, np.float32)
    rms_w = np.asarray(rms_w, np.float32)
    gamma = np.asarray(gamma, np.float32)
    w1f = np.asarray(w1f, np.float32)
    w3f = np.asarray(w3f, np.float32)
    w2f = np.asarray(w2f, np.float32)
    w1p = np.asarray(w1p, np.float32)
    w3p = np.asarray(w3p, np.float32)
    w2p = np.asarray(w2p, np.float32)
    n = x.shape[0]

    # ---- gate: softmax -> top-2 -> renormalize (host) ----
    logits = x @ Wg.T
    mx = logits.max(-1, keepdims=True)
    pr = np.exp(logits - mx)
    pr /= pr.sum(-1, keepdims=True)
    ti = np.argsort(-pr, axis=-1)[:, :TOPK]
    tw = np.take_along_axis(pr, ti, axis=-1)
    tw = tw / tw.sum(-1, keepdims=True)

    # token lists per expert (order: append over k slots then tokens)
    sel_tok = [[] for _ in range(E)]
    sel_w = [[] for _ in range(E)]
    for k in range(TOPK):
        col_e = ti[:, k]
        col_w = tw[:, k]
        for e in range(E):
            msk = col_e == e
            sel_tok[e].append(np.nonzero(msk)[0])
            sel_w[e].append(col_w[msk])
    sel_tok = [np.concatenate(s) for s in sel_tok]
    sel_w = [np.concatenate(s).astype(np.float32) for s in sel_w]
    counts = [len(s) for s in sel_tok]

    # ---- RMS norm core (host) ----
    y = x * (1.0 / np.sqrt((x * x).mean(-1, keepdims=True) + EPS))

    # ---- jobs: (kind, expert, h-chunk); fractal 2 chunks, plain 4 ----
    jobs = [("f", f, c) for f in range(F) for c in range(2)]
    jobs += [("p", p, c) for p in range(P) for c in range(4)]
    assert len(jobs) == N_CORES * UPC

    def job_eid(j):
        kind, e, _ = jobs[j]
        return e if kind == "f" else e + F

    # greedy 3-slots-per-core balance by token count
    order = sorted(range(len(jobs)), key=lambda j: -counts[job_eid(j)])
    loads = [0] * N_CORES
    slots = [[] for _ in range(N_CORES)]
    for j in order:
        for i in sorted(range(N_CORES), key=lambda i: loads[i]):
            if len(slots[i]) < UPC:
                slots[i].append(j)
                loads[i] += counts[job_eid(j)]
                break

    # ---- pack per-core inputs ----
    # overflow tokens beyond T_PAD are handled on the host (never expected
    # for the benchmark data where max count ~2175)
    in_maps = []
    for i in range(N_CORES):
        w1m = np.empty((UPC, D, HC), np.float32)
        w3m = np.empty((UPC, D, HC), np.float32)
        w2m = np.empty((UPC, HC, D), np.float32)
        xm = np.zeros((UPC, D, T_PAD), np.float32)
        for s, j in enumerate(slots[i]):
            kind, e, c = jobs[j]
            hs = slice(c * HC, (c + 1) * HC)
            eid = job_eid(j)
            toks = sel_tok[eid][:T_PAD]
            if kind == "f":
                w1m[s] = w1f[e][hs].T
                w3m[s] = w3f[e][hs].T
                w2m[s] = (w2f[e][:, hs] * gamma[e][:, None]).T
                xm[s, :, :len(toks)] = (y[toks] * rms_w[e]).T
            else:
                w1m[s] = w1p[e][hs].T
                w3m[s] = w3p[e][hs].T
                w2m[s] = w2p[e][:, hs].T
                xm[s, :, :len(toks)] = x[toks].T
        in_maps.append({"w1t": w1m, "w3t": w3m, "w2t": w2m, "xt": xm})

    # ---- run on the 8 NeuronCores ----
    nc = _get_compiled()
    trace = os.environ.get("BASS_KERNEL_TRACE", "0") == "1"
    res = bass_utils.run_bass_kernel_spmd(
        nc, in_maps, core_ids=list(range(N_CORES)), trace=trace
    )
    global _LAST_RESULTS
    _LAST_RESULTS = res

    # ---- host combine ----
    out = np.zeros((n, D), np.float32)
    # fractal residual terms: cw * (gamma*yn + x) for each selected pair
    for e in range(F):
        toks, ws = sel_tok[e], sel_w[e]
        yn = y[toks] * rms_w[e]
        out[toks] += ws[:, None] * (gamma[e] * yn + x[toks])
    # device unit outputs: cw * (W2c' @ swiglu-chunk)
    for i in range(N_CORES):
        uo = res.results[i]["out"]
        for s, j in enumerate(slots[i]):
            eid = job_eid(j)
            toks, ws = sel_tok[eid], sel_w[eid]
            tcap = min(len(toks), T_PAD)
            out[toks[:tcap]] += ws[:tcap, None] * uo[s, :, :tcap].T

    # host fallback for (never-expected) overflow tokens beyond T_PAD
    for eid in range(E):
        if counts[eid] <= T_PAD:
            continue
        toks = sel_tok[eid][T_PAD:]
        ws = sel_w[eid][T_PAD:]
        if eid < F:
            e = eid
            xin = y[toks] * rms_w[e]
            h = _np_silu(xin @ w1f[e].T) * (xin @ w3f[e].T)
            sf = h @ w2f[e].T
            out[toks] += ws[:, None] * (gamma[e] * sf)
        else:
            e = eid - F
            h = _np_silu(x[toks] @ w1p[e].T) * (x[toks] @ w3p[e].T)
            out[toks] += ws[:, None] * (h @ w2p[e].T)

    return out
